# revision 1
# baseline (speedup 1.0000x reference)
"""BlackwellLinear Trainium2 kernel: 2:4 sparsity + int8 fake-quant + x @ w.T + bias.

Full inputs in, full output out. Data-parallel over tokens across 8 NeuronCores;
weight/bias replicated. All module math (sparsify, quantize, matmul, bias) runs
on device; the host only re-encodes layouts: x is transposed, split into exact
fp16 hi/lo planes, and the in_features axis of both x.T and w.T is permuted
phase-major (p <-> 4*(p%256) + p//256). The permutation makes each group-of-4
(the 2:4 sparsity unit) span four k-tiles at the SAME partition/column
coordinates, so the whole sparsify+quantize pipeline is contiguous full-width
elementwise ops and the quantized weight is produced directly in [in_f, out_f]
(lhsT) layout -- no on-device transposes. A contraction-axis permutation
applied to both operands leaves the matmul result unchanged.

Numerics: the reference computes q = round(clip(w_sp / scale)) with
scale = absmax/127 in fp32. There is no float divide on the vector engine, so
the kernel reproduces fl-division bit-exactly (up to ~2^-30 probability edge
cases) with a reciprocal-multiply followed by an exact-residual correction:
  k  = rne(w * inv)                     (magic-constant RNE round)
  d  = (w - k*s_hi) - k*s_lo            (exact: k is a small integer, s split)
  q  = rne(k + d*inv)
clip is a no-op because |w_sp| <= absmax ==> |w_sp/scale| <= 127.00002 < 127.5.
The dequant scale is folded into the PSUM eviction (y = s*(x@q.T) + bias).

Matmul precision: q is an integer <= 127 so it is fp16-exact. x is split as
x = x_hi + x_lo with both halves fp16 (x_hi = fp16(x), x_lo = fp16(x - x_hi);
the residual subtract is exact by Sterbenz, total representation error
~2^-23 |x|). Products x_part * q fit in 18 bits -> exact, accumulated in fp32
PSUM. Two fp16 passes run at 1 cycle/row on the PE -- 2x native fp32 matmul
speed at fp32-envelope accuracy.
"""

import numpy as np

N_CORES = 8
P = 128
IN_F = 1024
OUT_F = 1024
TOKENS = 32768
TOK_PER_CORE = TOKENS // N_CORES  # 4096
K_TILES = IN_F // P  # 8
M_TILES = OUT_F // P  # 8
TB_TOK = 1024  # token block per x strip
N_TB = TOK_PER_CORE // TB_TOK  # 4
MM_N = 512  # matmul moving free dim (one PSUM bank of fp32)
TJ = TB_TOK // MM_N  # matmuls per (mi, ki, part) stationary load

MAGIC = 12582912.0  # 1.5 * 2**23: (v + MAGIC) - MAGIC == RNE round for |v| <= 2**22
SPLIT = 4097.0  # 2**12 + 1: Veltkamp split constant for fp32

# phase-major permutation of the in_features axis: position p holds original
# feature 4*(p%256) + p//256, so k-tile kt covers phase kt//2 of group range
# (kt%2)*128..+128 and the four phases of a group share partition/column coords
_PERM = (4 * (np.arange(IN_F) % 256) + np.arange(IN_F) // 256).astype(np.int64)

_CACHE = {}


def _build(qmax: float):
    from contextlib import ExitStack

    import concourse.tile as tile
    import concourse.mybir as mybir
    from concourse import bacc, bass_isa

    f32 = mybir.dt.float32
    f16 = mybir.dt.float16
    Alu = mybir.AluOpType
    Act = mybir.ActivationFunctionType

    inv_qmax = float(np.float32(1.0) / np.float32(qmax))
    qmaxf = float(np.float32(qmax))

    nc = bacc.Bacc("TRN2", target_bir_lowering=False, debug=False)
    xth = nc.dram_tensor("xth", [IN_F, TOK_PER_CORE], f16, kind="ExternalInput").ap()
    xtl = nc.dram_tensor("xtl", [IN_F, TOK_PER_CORE], f16, kind="ExternalInput").ap()
    # wp: w.T with permuted in_f rows = [in_f_perm, out_f], fp32
    wp = nc.dram_tensor("wp", [IN_F, OUT_F], f32, kind="ExternalInput").ap()
    bias = nc.dram_tensor("bias", [OUT_F], f32, kind="ExternalInput").ap()
    yt = nc.dram_tensor("yt", [OUT_F, TOK_PER_CORE], f32, kind="ExternalOutput").ap()

    with tile.TileContext(nc) as tc, ExitStack() as ctx:
        const = ctx.enter_context(tc.tile_pool(name="const", bufs=1))
        wnat_p = ctx.enter_context(tc.tile_pool(name="wnat", bufs=8))
        abs_p = ctx.enter_context(tc.tile_pool(name="absp", bufs=8))
        thr_p = ctx.enter_context(tc.tile_pool(name="thr", bufs=2))
        thrtmp_p = ctx.enter_context(tc.tile_pool(name="thrtmp", bufs=1))
        scratch = ctx.enter_context(tc.tile_pool(name="scratch", bufs=2))
        qtmp_p = ctx.enter_context(tc.tile_pool(name="qtmp", bufs=2))
        qtmp1_p = ctx.enter_context(tc.tile_pool(name="qtmp1", bufs=1))
        wqt_p = ctx.enter_context(tc.tile_pool(name="wqt", bufs=8))
        sc_p = ctx.enter_context(tc.tile_pool(name="sc", bufs=1))
        x_p = ctx.enter_context(tc.tile_pool(name="x", bufs=9))
        y_p = ctx.enter_context(tc.tile_pool(name="y", bufs=4))
        psum_mm = ctx.enter_context(tc.tile_pool(name="psmm", bufs=8, space="PSUM"))

        # ---- weight load (split across both HWDGE queues for full BW) ----
        wk = [None] * K_TILES
        ak = [None] * K_TILES
        cm = sc_p.tile([P, 8], f32, tag="cm")
        for i, kt in enumerate((0, 1, 2, 3, 4, 5, 6, 7)):
            wt = wnat_p.tile([P, OUT_F], f32, tag="wnat", name=f"wnat{kt}")
            (nc.sync if kt % 2 == 0 else nc.scalar).dma_start(
                wt[:], wp[kt * P : (kt + 1) * P, :]
            )
            wk[kt] = wt
            a = abs_p.tile([P, OUT_F], f32, tag="abs", name=f"abs{kt}")
            nc.scalar.activation(a[:], wt[:], Act.Abs)
            ak[kt] = a
            nc.vector.tensor_reduce(
                out=cm[:, kt : kt + 1],
                in_=a[:],
                axis=mybir.AxisListType.X,
                op=Alu.max,
            )

        # ---- global absmax broadcast to all partitions ----
        amc = sc_p.tile([P, 1], f32, tag="amc")
        nc.vector.reduce_max(amc[:], cm[:], axis=mybir.AxisListType.X)
        am = sc_p.tile([P, 1], f32, tag="am")
        nc.gpsimd.partition_all_reduce(
            am[:], amc[:], channels=P, reduce_op=bass_isa.ReduceOp.max
        )

        # ---- s = fl(absmax/qmax) bit-exact; split s; inv ~= 1/s ----
        _scn = [0]

        def sc_tile():
            _scn[0] += 1
            return sc_p.tile([P, 1], f32, tag=f"sct{_scn[0]}", name=f"sct{_scn[0]}")

        def vts(out, in0, s1, op0, s2=None, op1=None):
            kw = {"op1": op1} if op1 is not None else {}
            nc.vector.tensor_scalar(
                out=out, in0=in0, scalar1=s1, scalar2=s2, op0=op0, **kw
            )

        def vtt(out, in0, in1, op):
            nc.vector.tensor_tensor(out=out, in0=in0, in1=in1, op=op)

        sq0, sc_, stq, shi, slo = (sc_tile() for _ in range(5))
        su, sv, su2, sr, src = (sc_tile() for _ in range(5))
        s_t = sc_p.tile([P, 1], f32, tag="s")
        vts(sq0[:], am[:], inv_qmax, Alu.mult)
        vts(sc_[:], sq0[:], SPLIT, Alu.mult)
        vtt(stq[:], sc_[:], sq0[:], Alu.subtract)
        vtt(shi[:], sc_[:], stq[:], Alu.subtract)
        vtt(slo[:], sq0[:], shi[:], Alu.subtract)
        vts(su[:], shi[:], qmaxf, Alu.mult)
        vtt(sv[:], am[:], su[:], Alu.subtract)
        vts(su2[:], slo[:], qmaxf, Alu.mult)
        vtt(sr[:], sv[:], su2[:], Alu.subtract)
        vts(src[:], sr[:], inv_qmax, Alu.mult)
        vtt(s_t[:], sq0[:], src[:], Alu.add)

        s_hi = sc_p.tile([P, 1], f32, tag="shi")
        s_lo = sc_p.tile([P, 1], f32, tag="slo")
        scs, scts = sc_tile(), sc_tile()
        vts(scs[:], s_t[:], SPLIT, Alu.mult)
        vtt(scts[:], scs[:], s_t[:], Alu.subtract)
        vtt(s_hi[:], scs[:], scts[:], Alu.subtract)
        vtt(s_lo[:], s_t[:], s_hi[:], Alu.subtract)

        inv_t = sc_p.tile([P, 1], f32, tag="inv")
        r0 = sc_tile()
        nc.vector.reciprocal(r0[:], s_t[:])
        for _ in range(2):
            p1, e1, r1 = sc_tile(), sc_tile(), sc_tile()
            vtt(p1[:], s_t[:], r0[:], Alu.mult)
            vts(e1[:], p1[:], 2.0, Alu.subtract)  # p1 - 2 = -(2 - p1)
            vtt(r1[:], r0[:], e1[:], Alu.mult)
            vts(r0[:], r1[:], -1.0, Alu.mult)  # r0 * (2 - p1)
        nc.vector.tensor_copy(inv_t[:], r0[:])
        ninv_t = sc_p.tile([P, 1], f32, tag="ninv")
        vts(ninv_t[:], inv_t[:], -1.0, Alu.mult)
        magic_t = sc_p.tile([P, 1], f32, tag="magic")
        nc.gpsimd.memset(magic_t[:], MAGIC)
        nmagic_t = sc_p.tile([P, 1], f32, tag="nmagic")
        nc.gpsimd.memset(nmagic_t[:], -MAGIC)
        one_t = sc_p.tile([P, 1], f32, tag="one")
        nc.gpsimd.memset(one_t[:], 1.0)

        # ---- bias slices ----
        bias_t = []
        for mi in range(M_TILES):
            bt = const.tile([P, 1], f32, tag=f"bias{mi}")
            nc.sync.dma_start(bt[:, 0:1], bias[mi * P : (mi + 1) * P].unsqueeze(1))
            bias_t.append(bt)

        # ---- 2:4 threshold per group-range (contiguous, phases = k-tiles) ----
        # thr_r = 2nd largest |w| of each group = max(min of pair maxes,
        # max of pair mins) over the 4 phase tiles of range r
        def build_thr(r):
            a0, a1, a2, a3 = (ak[2 * j + r] for j in range(4))
            tA = thrtmp_p.tile([P, OUT_F], f32, tag="tA", name=f"tA_{r}")
            tB = thrtmp_p.tile([P, OUT_F], f32, tag="tB", name=f"tB_{r}")
            tC = thrtmp_p.tile([P, OUT_F], f32, tag="tC", name=f"tC_{r}")
            tr = thr_p.tile([P, OUT_F], f32, tag="thr", name=f"thr_{r}")
            vtt(tA[:], a0[:], a1[:], Alu.max)
            vtt(tB[:], a2[:], a3[:], Alu.max)
            vtt(tA[:], tA[:], tB[:], Alu.min)  # t1 = min of pair maxes
            vtt(tB[:], a0[:], a1[:], Alu.min)
            vtt(tC[:], a2[:], a3[:], Alu.min)
            vtt(tB[:], tB[:], tC[:], Alu.max)  # t2 = max of pair mins
            vtt(tr[:], tA[:], tB[:], Alu.max)
            return tr

        # ---- per k-tile: quantize w directly (rounding commutes with the
        # sparsity mask elementwise), mask in parallel, combine at the end.
        # q16 k-tiles land directly in lhsT [in_f, out_f] layout.
        # emission order drives Tile's scheduling priority: put k-tile 0's
        # whole chain (thr range 0 -> quant -> mask) ahead of everything else
        # so the PE's first stationary tile lands as early as possible
        wqt_by_kt = {}
        thr_cache = {}
        kt_order = list(range(K_TILES))
        for kt in kt_order:
            r = kt % 2
            if r not in thr_cache:
                thr_cache[r] = build_thr(r)
            wt, a, tr = wk[kt], ak[kt], thr_cache[r]
            m = scratch.tile([P, OUT_F], f32, tag="mask")
            vtt(m[:], a[:], tr[:], Alu.is_ge)

            q0 = qtmp_p.tile([P, OUT_F], f32, tag="q0")
            k = qtmp_p.tile([P, OUT_F], f32, tag="k")
            n1 = qtmp1_p.tile([P, OUT_F], f32, tag="n1")
            n2 = qtmp1_p.tile([P, OUT_F], f32, tag="n2")
            # k = rne(w * inv) via the magic constant (ACT: in*scale + bias)
            nc.scalar.activation(
                q0[:], wt[:], Act.Identity, bias=magic_t[:], scale=inv_t[:]
            )
            nc.scalar.activation(
                k[:], q0[:], Act.Identity, bias=nmagic_t[:], scale=one_t[:]
            )
            # exact residual: n2 = k*s - w (k integer, s split => exact)
            nc.vector.scalar_tensor_tensor(
                out=n1[:], in0=k[:], scalar=s_hi[:], in1=wt[:],
                op0=Alu.mult, op1=Alu.subtract,
            )
            nc.vector.scalar_tensor_tensor(
                out=n2[:], in0=k[:], scalar=s_lo[:], in1=n1[:],
                op0=Alu.mult, op1=Alu.add,
            )
            # v = k + (w - k*s)*inv = k + n2*(-inv)
            nc.vector.scalar_tensor_tensor(
                out=q0[:], in0=n2[:], scalar=ninv_t[:], in1=k[:],
                op0=Alu.mult, op1=Alu.add,
            )
            vts(q0[:], q0[:], MAGIC, Alu.add, MAGIC, Alu.subtract)  # q = rne(v)
            vtt(q0[:], q0[:], m[:], Alu.mult)  # apply 2:4 mask
            q16 = wqt_p.tile([P, OUT_F], f16, tag="q16", name=f"q16_{kt}")
            nc.scalar.copy(q16[:], q0[:])
            wqt_by_kt[kt] = q16
        wqt = [wqt_by_kt[kt] for kt in range(K_TILES)]

        # ---- main matmul: yt[m, t] = sum_k wqt[k,m].T @ (xh[k,t] + xl[k,t]) ----
        # tb0 x loads share the sync queue (after w); later tbs go on the ACT
        # queue and self-throttle via pool backpressure; out stores on ACT queue
        for tb in range(N_TB):
            dma_eng = nc.sync if tb == 0 else nc.scalar
            xh, xl = [], []
            for ki in range(K_TILES):
                sl_p = slice(ki * P, (ki + 1) * P)
                sl_t = slice(tb * TB_TOK, (tb + 1) * TB_TOK)
                xht = x_p.tile([P, TB_TOK], f16, tag="xh", name=f"xh{tb}_{ki}")
                dma_eng.dma_start(xht[:], xth[sl_p, sl_t])
                xlt = x_p.tile([P, TB_TOK], f16, tag="xl", name=f"xl{tb}_{ki}")
                dma_eng.dma_start(xlt[:], xtl[sl_p, sl_t])
                xh.append(xht)
                xl.append(xlt)
            def evict(mi, ps_tj):
                for tj in range(TJ):
                    ysb = y_p.tile([P, MM_N], f32, tag="ysb", name=f"y{tb}_{mi}_{tj}")
                    nc.scalar.activation(
                        ysb[:],
                        ps_tj[tj][:],
                        Act.Identity,
                        bias=bias_t[mi][:],
                        scale=s_t[:],
                    )
                    tcol = tb * TB_TOK + tj * MM_N
                    nc.scalar.dma_start(
                        yt[mi * P : (mi + 1) * P, tcol : tcol + MM_N], ysb[:]
                    )

            if tb == 0:
                # k-outer sweep: PE starts as soon as the first quantized
                # k-tile lands, consuming k-tiles at the prep pipeline's pace
                for mh in range(2):
                    ps = {
                        (ml, tj): psum_mm.tile(
                            [P, MM_N], f32, tag="ps", name=f"ps0_{mh}_{ml}_{tj}"
                        )
                        for ml in range(4)
                        for tj in range(TJ)
                    }
                    for ki in range(K_TILES):
                        for ml in range(4):
                            mi = mh * 4 + ml
                            lhsT = wqt[ki][:, mi * P : (mi + 1) * P]
                            for part, xp in ((0, xh), (1, xl)):
                                for tj in range(TJ):
                                    nc.tensor.matmul(
                                        ps[ml, tj][:],
                                        lhsT,
                                        xp[ki][:, tj * MM_N : (tj + 1) * MM_N],
                                        start=(ki == 0 and part == 0),
                                        stop=(ki == K_TILES - 1 and part == 1),
                                    )
                    for ml in range(4):
                        evict(mh * 4 + ml, [ps[ml, tj] for tj in range(TJ)])
            else:
                for mi in range(M_TILES):
                    ps = [
                        psum_mm.tile(
                            [P, MM_N], f32, tag="ps", name=f"ps{tb}_{mi}_{tj}"
                        )
                        for tj in range(TJ)
                    ]
                    for ki in range(K_TILES):
                        lhsT = wqt[ki][:, mi * P : (mi + 1) * P]
                        for part, xp in ((0, xh), (1, xl)):
                            for tj in range(TJ):
                                nc.tensor.matmul(
                                    ps[tj][:],
                                    lhsT,
                                    xp[ki][:, tj * MM_N : (tj + 1) * MM_N],
                                    start=(ki == 0 and part == 0),
                                    stop=(ki == K_TILES - 1 and part == 1),
                                )
                    evict(mi, ps)

    nc.compile()
    return nc


def _get(qmax: float):
    key = qmax
    if key not in _CACHE:
        _CACHE[key] = _build(qmax)
    return _CACHE[key]


def host_prep(x, weight):
    """Host-side input re-encoding: transpose, phase-major permute the in_f
    axis, exact fp16 hi/lo split of x. Pure layout/encoding; no module math."""
    xt = np.ascontiguousarray(x.T)[_PERM]  # [IN_F perm, TOKENS]
    xth = xt.astype(np.float16)
    xtl = (xt - xth.astype(np.float32)).astype(np.float16)
    wp = np.ascontiguousarray(weight.T[_PERM])  # [IN_F perm, OUT_F]
    return xth, xtl, wp


LAST_EXEC_NS = None


def kernel(x, weight, bias, precision, _trace_dir=None):
    global LAST_EXEC_NS
    from concourse.bass_utils import run_bass_kernel_spmd

    x = np.asarray(x, dtype=np.float32)
    weight = np.asarray(weight, dtype=np.float32)
    bias = np.asarray(bias, dtype=np.float32)
    prec = int(np.asarray(precision))
    qmax = float(2 ** (prec - 1) - 1)

    nc = _get(qmax)

    xth, xtl, wp = host_prep(x, weight)
    in_maps = [
        {
            "xth": np.ascontiguousarray(
                xth[:, c * TOK_PER_CORE : (c + 1) * TOK_PER_CORE]
            ),
            "xtl": np.ascontiguousarray(
                xtl[:, c * TOK_PER_CORE : (c + 1) * TOK_PER_CORE]
            ),
            "wp": wp,
            "bias": bias,
        }
        for c in range(N_CORES)
    ]
    kw = {}
    if _trace_dir is not None:
        kw = {"trace": True, "tmpdir": _trace_dir}
    res = run_bass_kernel_spmd(nc, in_maps, list(range(N_CORES)), **kw)
    LAST_EXEC_NS = res.exec_time_ns
    yt = np.concatenate([res.results[c]["yt"] for c in range(N_CORES)], axis=1)
    return np.ascontiguousarray(yt.T)



# revision 2
# speedup vs baseline: 1.8771x; 1.8771x over previous
"""BlackwellLinear Trainium2 kernel: 2:4 sparsity + int8 fake-quant + x @ w.T + bias.

Full inputs in, full output out. Data-parallel over tokens across 8 NeuronCores;
weight/bias replicated. All module math (sparsify, quantize, matmul, bias) runs
on device; the host only re-encodes layouts: x is transposed to fp16, and the
in_features axis of both x.T and w.T is permuted phase-major
(p <-> 4*(p%256) + p//256). The permutation makes each group-of-4 (the 2:4
sparsity unit) span four k-tiles at the SAME partition/column coordinates, so
the whole sparsify+quantize pipeline is contiguous full-width elementwise ops
and the quantized weight is produced directly in [in_f, out_f] (lhsT) layout --
no on-device transposes. A contraction-axis permutation applied to both
operands leaves the matmul result unchanged.

Numerics (target rel-err 2e-2; this achieves ~5e-4):
  q  = rne(w * inv) * mask         inv ~= 1/s via reciprocal + 1 NR step;
                                   rne via the +/- 1.5*2^23 magic constant.
  y  = s * (x16 @ q.T) + bias      x16 = fp16(x)  (2^-11 relative error),
                                   fp32 PSUM accumulate, fp16 store-out.
The 2:4 threshold compare stays fp32 (fp16 ties would keep >2 weights per
group). clip is a no-op: |w_sp| <= absmax ==> |w*inv| <= 127.00003 < 127.5.
q is an integer <= 127 so fp16 is exact; x16*q products are exact in fp32.

Schedule (per core: 4096 tokens, single fp16 matmul pass = 512 N=512 matmuls
~= 109us of PE):
  - w k-tiles DMA first (both HWDGE queues), abs+absmax reduced per-tile,
    global scale s + inv, then per-k-tile quantization, evens before odds so
    the PE can start consuming q16 tiles in order 0,2,4,6,1,3,5,7.
  - Phase A (tokens 0:512, k-outer, all 8 PSUM banks): starts as soon as
    q16[0] + a 512-token sliver of x land; consumes q16 tiles at the prep
    pipeline's pace.
  - Phase B1/B2 (tokens 512:2560, 2560:4096, m-outer, 4/3 banks per m-tile):
    x fully SBUF-resident, stationary weight reused across 4/3 matmuls,
    ACT evicts s*psum+bias directly to fp16 while the PE rolls on.
"""

import numpy as np

N_CORES = 8
P = 128
IN_F = 1024
OUT_F = 1024
TOKENS = 32768
TOK_PER_CORE = TOKENS // N_CORES  # 4096
K_TILES = IN_F // P  # 8
M_TILES = OUT_F // P  # 8
MM_N = 512  # matmul moving free dim (one PSUM bank of fp32)

TOK_A = 512  # phase A tokens (k-outer sweep, 8 banks = one per m-tile)
TOK_B1 = 2048  # phase B1 tokens (m-outer, 4 banks)
TOK_B2 = TOK_PER_CORE - TOK_A - TOK_B1  # 1536 (m-outer, 3 banks)

MAGIC = 12582912.0  # 1.5 * 2**23: (v + MAGIC) - MAGIC == RNE round for |v| <= 2**22

# order in which k-tiles are produced/consumed: thr range 0 (even k-tiles)
# is ready first, range 1 (odd) later
KORD = (0, 2, 4, 6, 1, 3, 5, 7)

# phase-major permutation of the in_features axis: position p holds original
# feature 4*(p%256) + p//256, so k-tile kt covers phase kt//2 of group range
# (kt%2)*128..+128 and the four phases of a group share partition/column coords
_PERM = (4 * (np.arange(IN_F) % 256) + np.arange(IN_F) // 256).astype(np.int64)

_CACHE = {}


def _build(qmax: float):
    from contextlib import ExitStack

    import concourse.tile as tile
    import concourse.mybir as mybir
    from concourse import bacc, bass_isa

    f32 = mybir.dt.float32
    f16 = mybir.dt.float16
    Alu = mybir.AluOpType
    Act = mybir.ActivationFunctionType

    inv_qmax = float(np.float32(1.0) / np.float32(qmax))

    nc = bacc.Bacc("TRN2", target_bir_lowering=False, debug=False)
    xt16 = nc.dram_tensor("xt16", [IN_F, TOK_PER_CORE], f16, kind="ExternalInput").ap()
    # wp: w.T with permuted in_f rows = [in_f_perm, out_f], fp32
    wp = nc.dram_tensor("wp", [IN_F, OUT_F], f32, kind="ExternalInput").ap()
    bias = nc.dram_tensor("bias", [OUT_F], f32, kind="ExternalInput").ap()
    yt = nc.dram_tensor("yt", [OUT_F, TOK_PER_CORE], f16, kind="ExternalOutput").ap()

    with tile.TileContext(nc) as tc, ExitStack() as ctx:
        const = ctx.enter_context(tc.tile_pool(name="const", bufs=1))
        wnat_p = ctx.enter_context(tc.tile_pool(name="wnat", bufs=8))
        abs_p = ctx.enter_context(tc.tile_pool(name="absp", bufs=8))
        thr_p = ctx.enter_context(tc.tile_pool(name="thr", bufs=2))
        thrtmp_p = ctx.enter_context(tc.tile_pool(name="thrtmp", bufs=2))
        m_p = ctx.enter_context(tc.tile_pool(name="mask", bufs=2))
        q0_p = ctx.enter_context(tc.tile_pool(name="q0", bufs=2))
        q16_p = ctx.enter_context(tc.tile_pool(name="q16", bufs=8))
        sc_p = ctx.enter_context(tc.tile_pool(name="sc", bufs=1))
        xa_p = ctx.enter_context(tc.tile_pool(name="xa", bufs=8))
        xb_p = ctx.enter_context(tc.tile_pool(name="xb", bufs=8))
        ya_p = ctx.enter_context(tc.tile_pool(name="ya", bufs=4))
        yb_p = ctx.enter_context(tc.tile_pool(name="yb", bufs=2))
        psum_mm = ctx.enter_context(tc.tile_pool(name="psmm", bufs=8, space="PSUM"))

        def vts(out, in0, s1, op0, s2=None, op1=None):
            kw = {"op1": op1} if op1 is not None else {}
            nc.vector.tensor_scalar(
                out=out, in0=in0, scalar1=s1, scalar2=s2, op0=op0, **kw
            )

        def vtt(out, in0, in1, op):
            nc.vector.tensor_tensor(out=out, in0=in0, in1=in1, op=op)

        # ---- weight load (split across both HWDGE queues, evens first) ----
        wk = [None] * K_TILES
        ak = [None] * K_TILES
        cm = sc_p.tile([P, 8], f32, tag="cm")
        for eng, kts in ((nc.sync, (0, 2, 1, 3)), (nc.scalar, (4, 6, 5, 7))):
            for kt in kts:
                wt = wnat_p.tile([P, OUT_F], f32, tag="wnat", name=f"wnat{kt}")
                eng.dma_start(wt[:], wp[kt * P : (kt + 1) * P, :])
                wk[kt] = wt

        # ---- bias slices (tiny) ----
        bias_t = []
        for mi in range(M_TILES):
            bt = const.tile([P, 1], f32, tag=f"bias{mi}")
            nc.sync.dma_start(bt[:, 0:1], bias[mi * P : (mi + 1) * P].unsqueeze(1))
            bias_t.append(bt)

        # ---- per-tile |w| and absmax column ----
        for pos, kt in enumerate(KORD):
            a = abs_p.tile([P, OUT_F], f32, tag="abs", name=f"abs{kt}")
            nc.scalar.activation(a[:], wk[kt][:], Act.Abs)
            ak[kt] = a
            nc.vector.tensor_reduce(
                out=cm[:, pos : pos + 1],
                in_=a[:],
                axis=mybir.AxisListType.X,
                op=Alu.max,
            )

        # ---- global absmax -> s = absmax/qmax, inv ~= 1/s ----
        amc = sc_p.tile([P, 1], f32, tag="amc")
        nc.vector.tensor_reduce(
            out=amc[:], in_=cm[:], axis=mybir.AxisListType.X, op=Alu.max
        )
        am = sc_p.tile([P, 1], f32, tag="am")
        nc.gpsimd.partition_all_reduce(
            am[:], amc[:], channels=P, reduce_op=bass_isa.ReduceOp.max
        )
        s_t = sc_p.tile([P, 1], f32, tag="s")
        vts(s_t[:], am[:], inv_qmax, Alu.mult)
        # reciprocal + one Newton-Raphson step: rel err ~2^-22 -> a handful of
        # +/-1 flips of q across the whole weight (harmless at 2e-2 tolerance)
        r0 = sc_p.tile([P, 1], f32, tag="r0")
        nc.vector.reciprocal(r0[:], s_t[:])
        p1 = sc_p.tile([P, 1], f32, tag="p1")
        e1 = sc_p.tile([P, 1], f32, tag="e1")
        r1 = sc_p.tile([P, 1], f32, tag="r1")
        inv_t = sc_p.tile([P, 1], f32, tag="inv")
        vtt(p1[:], s_t[:], r0[:], Alu.mult)
        vts(e1[:], p1[:], 2.0, Alu.subtract)  # p1 - 2 = -(2 - p1)
        vtt(r1[:], r0[:], e1[:], Alu.mult)
        vts(inv_t[:], r1[:], -1.0, Alu.mult)  # r0 * (2 - p1)
        magic_t = sc_p.tile([P, 1], f32, tag="magic")
        nc.gpsimd.memset(magic_t[:], MAGIC)
        nmagic_t = sc_p.tile([P, 1], f32, tag="nmagic")
        nc.gpsimd.memset(nmagic_t[:], -MAGIC)
        one_t = sc_p.tile([P, 1], f32, tag="one")
        nc.gpsimd.memset(one_t[:], 1.0)

        # ---- 2:4 threshold per group-range (fp32 compare; phases = k-tiles):
        # thr_r = 2nd largest |w| of each group = max(min of pair maxes,
        # max of pair mins) over the 4 phase tiles of range r
        def build_thr(r):
            a0, a1, a2, a3 = (ak[2 * j + r] for j in range(4))
            tA = thrtmp_p.tile([P, OUT_F], f32, tag="tA", name=f"tA_{r}")
            tB = thrtmp_p.tile([P, OUT_F], f32, tag="tB", name=f"tB_{r}")
            tr = thr_p.tile([P, OUT_F], f32, tag="thr", name=f"thr_{r}")
            vtt(tA[:], a0[:], a1[:], Alu.max)
            vtt(tB[:], a2[:], a3[:], Alu.max)
            vtt(tA[:], tA[:], tB[:], Alu.min)  # t1 = min of pair maxes
            vtt(tB[:], a0[:], a1[:], Alu.min)
            vtt(tr[:], a2[:], a3[:], Alu.min)
            vtt(tB[:], tB[:], tr[:], Alu.max)  # t2 = max of pair mins
            vtt(tr[:], tA[:], tB[:], Alu.max)
            return tr

        # ---- quantize per k-tile, evens then odds; q16 lands in lhsT layout
        wqt = [None] * K_TILES
        thr_cache = {}
        for kt in KORD:
            r = kt % 2
            if r not in thr_cache:
                thr_cache[r] = build_thr(r)
            wt, a, tr = wk[kt], ak[kt], thr_cache[r]
            m16 = m_p.tile([P, OUT_F], f16, tag="mask")
            vtt(m16[:], a[:], tr[:], Alu.is_ge)  # 0.0/1.0, exact in fp16
            q0 = q0_p.tile([P, OUT_F], f32, tag="q0")
            # k = rne(w * inv) via the magic constant (ACT: in*scale + bias)
            nc.scalar.activation(
                q0[:], wt[:], Act.Identity, bias=magic_t[:], scale=inv_t[:]
            )
            q16u = q0_p.tile([P, OUT_F], f16, tag="q16u")
            nc.scalar.activation(
                q16u[:], q0[:], Act.Identity, bias=nmagic_t[:], scale=one_t[:]
            )
            q16 = q16_p.tile([P, OUT_F], f16, tag="q16", name=f"q16_{kt}")
            vtt(q16[:], q16u[:], m16[:], Alu.mult)  # apply 2:4 mask
            wqt[kt] = q16

        # ---- x loads: phase-A sliver first, then the rest; evens first ----
        xa = [None] * K_TILES
        xb = [None] * K_TILES
        for eng, kts in ((nc.sync, (0, 2, 1, 3)), (nc.scalar, (4, 6, 5, 7))):
            for kt in kts:
                sl_p = slice(kt * P, (kt + 1) * P)
                t = xa_p.tile([P, TOK_A], f16, tag="xa", name=f"xa{kt}")
                eng.dma_start(t[:], xt16[sl_p, 0:TOK_A])
                xa[kt] = t
        for eng, kts in ((nc.sync, (0, 2, 1, 3)), (nc.scalar, (4, 6, 5, 7))):
            for kt in kts:
                sl_p = slice(kt * P, (kt + 1) * P)
                t = xb_p.tile([P, TOK_PER_CORE - TOK_A], f16, tag="xb", name=f"xb{kt}")
                eng.dma_start(t[:], xt16[sl_p, TOK_A:TOK_PER_CORE])
                xb[kt] = t

        # ---- phase A: tokens 0:512, k-outer over all 8 PSUM banks ----
        psA = [
            psum_mm.tile([P, MM_N], f32, tag="ps", name=f"psA_{mi}")
            for mi in range(M_TILES)
        ]
        for pos, kt in enumerate(KORD):
            for mi in range(M_TILES):
                nc.tensor.matmul(
                    psA[mi][:],
                    wqt[kt][:, mi * P : (mi + 1) * P],
                    xa[kt][:],
                    start=(pos == 0),
                    stop=(pos == K_TILES - 1),
                )
        for mi in range(M_TILES):
            ya = ya_p.tile([P, TOK_A], f16, tag="ya", name=f"yA_{mi}")
            nc.scalar.activation(
                ya[:], psA[mi][:], Act.Identity, bias=bias_t[mi][:], scale=s_t[:]
            )
            nc.sync.dma_start(yt[mi * P : (mi + 1) * P, 0:TOK_A], ya[:])

        # ---- phases B1/B2: m-outer, stationary weight reused across banks ----
        for phase, (col0, ncols) in enumerate(
            ((TOK_A, TOK_B1), (TOK_A + TOK_B1, TOK_B2))
        ):
            ntj = ncols // MM_N
            for mi in range(M_TILES):
                ps = [
                    psum_mm.tile([P, MM_N], f32, tag="ps", name=f"psB{phase}_{mi}_{tj}")
                    for tj in range(ntj)
                ]
                for pos, kt in enumerate(KORD):
                    lhsT = wqt[kt][:, mi * P : (mi + 1) * P]
                    xoff = col0 - TOK_A
                    for tj in range(ntj):
                        nc.tensor.matmul(
                            ps[tj][:],
                            lhsT,
                            xb[kt][:, xoff + tj * MM_N : xoff + (tj + 1) * MM_N],
                            start=(pos == 0),
                            stop=(pos == K_TILES - 1),
                        )
                yb = yb_p.tile([P, ncols], f16, tag="yb", name=f"yB{phase}_{mi}")
                for tj in range(ntj):
                    nc.scalar.activation(
                        yb[:, tj * MM_N : (tj + 1) * MM_N],
                        ps[tj][:],
                        Act.Identity,
                        bias=bias_t[mi][:],
                        scale=s_t[:],
                    )
                eng = nc.sync if mi % 2 == 0 else nc.scalar
                eng.dma_start(
                    yt[mi * P : (mi + 1) * P, col0 : col0 + ncols], yb[:]
                )

    nc.compile()
    return nc


def _get(qmax: float):
    key = qmax
    if key not in _CACHE:
        _CACHE[key] = _build(qmax)
    return _CACHE[key]


def host_prep(x, weight):
    """Host-side input re-encoding: transpose, phase-major permute the in_f
    axis, fp16 cast of x. Pure layout/encoding; no module math."""
    xt16 = np.ascontiguousarray(x.T)[_PERM].astype(np.float16)
    wp = np.ascontiguousarray(weight.T[_PERM])  # [IN_F perm, OUT_F]
    return xt16, wp


LAST_EXEC_NS = None


def kernel(x, weight, bias, precision, _trace_dir=None):
    global LAST_EXEC_NS
    from concourse.bass_utils import run_bass_kernel_spmd

    x = np.asarray(x, dtype=np.float32)
    weight = np.asarray(weight, dtype=np.float32)
    bias = np.asarray(bias, dtype=np.float32)
    prec = int(np.asarray(precision))
    qmax = float(2 ** (prec - 1) - 1)

    nc = _get(qmax)

    xt16, wp = host_prep(x, weight)
    in_maps = [
        {
            "xt16": np.ascontiguousarray(
                xt16[:, c * TOK_PER_CORE : (c + 1) * TOK_PER_CORE]
            ),
            "wp": wp,
            "bias": bias,
        }
        for c in range(N_CORES)
    ]
    kw = {}
    if _trace_dir is not None:
        kw = {"trace": True, "tmpdir": _trace_dir}
    res = run_bass_kernel_spmd(nc, in_maps, list(range(N_CORES)), **kw)
    LAST_EXEC_NS = res.exec_time_ns
    yt = np.concatenate([res.results[c]["yt"] for c in range(N_CORES)], axis=1)
    return np.ascontiguousarray(yt.T).astype(np.float32)


# revision 4
# speedup vs baseline: 2.2086x; 1.1766x over previous
"""BlackwellLinear Trainium2 kernel: 2:4 sparsity + int8 fake-quant + x @ w.T + bias.

Full inputs in, full output out. Data-parallel over tokens across 8 NeuronCores;
weight/bias replicated. All module math (sparsify, quantize, matmul, bias) runs
on device; the host only re-encodes layouts: x is transposed to fp16 and both
x.T and w.T get (a) a phase-major permutation of the in_features axis and (b) a
partition-major DRAM layout. The permutation p -> 4*(128*(p//512) + p%128) +
(p//128)%4 makes each group-of-4 (the 2:4 sparsity unit) span the four k-tiles
of one contiguous k-tile block (range 0 = k-tiles 0-3, range 1 = k-tiles 4-7)
at the SAME partition/column coordinates, so sparsify+quantize is all
contiguous full-width elementwise ops and the quantized weight lands directly
in [in_f, out_f] (lhsT) layout. A contraction-axis permutation applied to both
operands leaves the matmul unchanged. The partition-major DRAM layout makes
every load a few large fully-contiguous-per-partition DMAs.

Numerics (target rel-err 2e-2; this achieves ~1e-3):
  q  = rne(w * inv) * mask         inv ~= 1/s via reciprocal + 1 NR step;
                                   rne via the +/- 1.5*2^23 magic constant.
  y  = s * (x16 @ q.T) + bias      x16 = fp16(x), fp32 PSUM accumulate,
                                   fp16 store-out.
The 2:4 threshold compare stays fp32 (fp16 would create ties that keep >2
weights per group). clip is a no-op: |w*inv| <= 127.00003 < 127.5. q is an
integer <= 127 so fp16 is exact; x16*q products are exact in fp32.

Startup critical path (the matmul stream itself is ~133us at the observed
2.0 GHz PE clock, so everything else must hide under/ahead of it):
  w (4 x 1MB DMAs, 2 queues) -> per-chunk absmax (DVE tensor_reduce with
  apply_absolute_value) -> gpsimd partition all-reduce -> s, inv -> quantize
  k-tile 0 -> first matmul at ~30us. Engine split chosen so no single engine
  serializes the prep: DVE does reduces + max-side of the 2:4 threshold +
  masks + final f16 mask-multiply, GPSIMD does the min-side + q0 of k-tiles
  4-7 + bias DMAs (SWDGE), ACT does |w| + the magic-constant rounding.
  Phase A (tokens 0:512, k-outer over all 8 PSUM banks) consumes q16 tiles
  one at a time at the prep pipeline's pace; phases P1/P2 are m-outer with
  4-matmul stationary-weight reuse and ACT eviction into fp16 tiles.
"""

import numpy as np

N_CORES = 8
P = 128
IN_F = 1024
OUT_F = 1024
TOKENS = 32768
TOK_PER_CORE = TOKENS // N_CORES  # 4096
K_TILES = IN_F // P  # 8
M_TILES = OUT_F // P  # 8
MM_N = 512  # matmul moving free dim (one PSUM bank of fp32)

TOK_A = 512  # phase A (k-outer sweep, one PSUM bank per m-tile)
TOK_H = 2048  # x half-size per k-tile tile

MAGIC = 12582912.0  # 1.5 * 2**23: (v + MAGIC) - MAGIC == RNE round for |v| <= 2**22

# phase-major permutation with contiguous ranges: position p holds original
# feature 4*(128*(p//512) + p%128) + (p//128)%4, so range r = p//512 occupies
# k-tiles 4r..4r+3 (one per phase) and the four phases of a group share
# partition/column coordinates
_AR = np.arange(IN_F)
_PERM = (4 * (128 * (_AR // 512) + _AR % 128) + (_AR // 128) % 4).astype(np.int64)

_CACHE = {}


def _build(qmax: float):
    from contextlib import ExitStack

    import concourse.tile as tile
    import concourse.mybir as mybir
    from concourse import bacc, bass_isa

    f32 = mybir.dt.float32
    f16 = mybir.dt.float16
    Alu = mybir.AluOpType
    Act = mybir.ActivationFunctionType

    inv_qmax = float(np.float32(1.0) / np.float32(qmax))

    nc = bacc.Bacc("TRN2", target_bir_lowering=False, debug=False)
    # partition-major: row p holds the 8 k-tile slices for partition p
    xt16 = nc.dram_tensor(
        "xt16", [P, K_TILES, TOK_PER_CORE], f16, kind="ExternalInput"
    ).ap()
    wp = nc.dram_tensor("wp", [P, K_TILES * OUT_F], f32, kind="ExternalInput").ap()
    bias = nc.dram_tensor("bias", [OUT_F], f32, kind="ExternalInput").ap()
    yt = nc.dram_tensor("yt", [OUT_F, TOK_PER_CORE], f16, kind="ExternalOutput").ap()

    with tile.TileContext(nc) as tc, ExitStack() as ctx:
        const = ctx.enter_context(tc.tile_pool(name="const", bufs=1))
        wch_p = ctx.enter_context(tc.tile_pool(name="wch", bufs=4))
        abs_p = ctx.enter_context(tc.tile_pool(name="absp", bufs=8))
        thr_p = ctx.enter_context(tc.tile_pool(name="thr", bufs=2))
        tmax_p = ctx.enter_context(tc.tile_pool(name="tmax", bufs=1))
        tmin_p = ctx.enter_context(tc.tile_pool(name="tmin", bufs=1))
        m_p = ctx.enter_context(tc.tile_pool(name="mask", bufs=2))
        q0_p = ctx.enter_context(tc.tile_pool(name="q0", bufs=2))
        q16_p = ctx.enter_context(tc.tile_pool(name="q16", bufs=8))
        sc_p = ctx.enter_context(tc.tile_pool(name="sc", bufs=1))
        x_p = ctx.enter_context(tc.tile_pool(name="x", bufs=16))
        ya_p = ctx.enter_context(tc.tile_pool(name="ya", bufs=2))
        yb_p = ctx.enter_context(tc.tile_pool(name="yb", bufs=2))
        psum_mm = ctx.enter_context(tc.tile_pool(name="psmm", bufs=8, space="PSUM"))

        def vts(out, in0, s1, op0, s2=None, op1=None):
            kw = {"op1": op1} if op1 is not None else {}
            nc.vector.tensor_scalar(
                out=out, in0=in0, scalar1=s1, scalar2=s2, op0=op0, **kw
            )

        def vtt(out, in0, in1, op):
            nc.vector.tensor_tensor(out=out, in0=in0, in1=in1, op=op)

        # ---- weight load: 4 x 1MB (k-tile pairs), both HWDGE queues ----
        wch = []
        for ci, eng in enumerate((nc.sync, nc.sync, nc.scalar, nc.scalar)):
            wt = wch_p.tile([P, 2 * OUT_F], f32, tag="wch", name=f"wch{ci}")
            eng.dma_start(wt[:], wp[:, ci * 2 * OUT_F : (ci + 1) * 2 * OUT_F])
            wch.append(wt)

        def wk(kt):  # [P, OUT_F] view of k-tile kt
            return wch[kt // 2][:, (kt % 2) * OUT_F : (kt % 2 + 1) * OUT_F]

        # ---- bias slices on SWDGE (keeps the HWDGE load queues clean) ----
        bias_t = []
        for mi in range(M_TILES):
            bt = const.tile([P, 1], f32, tag=f"bias{mi}")
            nc.gpsimd.dma_start(bt[:, 0:1], bias[mi * P : (mi + 1) * P].unsqueeze(1))
            bias_t.append(bt)
        magic_t = sc_p.tile([P, 1], f32, tag="magic")
        nc.gpsimd.memset(magic_t[:], MAGIC)
        nmagic_t = sc_p.tile([P, 1], f32, tag="nmagic")
        nc.gpsimd.memset(nmagic_t[:], -MAGIC)
        one_t = sc_p.tile([P, 1], f32, tag="one")
        nc.gpsimd.memset(one_t[:], 1.0)
        z16 = sc_p.tile([P, P], f16, tag="z16")
        nc.gpsimd.memset(z16[:], 0.0)

        # ---- global absmax: one |.|-max reduce per 1MB chunk (DVE) ----
        cm = sc_p.tile([P, 4], f32, tag="cm")
        for ci in range(4):
            nc.vector.tensor_reduce(
                out=cm[:, ci : ci + 1],
                in_=wch[ci][:],
                axis=mybir.AxisListType.X,
                op=Alu.max,
                apply_absolute_value=True,
            )

        # ---- |w| tiles (ACT) in k-tile order ----
        ak = []
        for kt in range(K_TILES):
            a = abs_p.tile([P, OUT_F], f32, tag="abs", name=f"abs{kt}")
            nc.scalar.activation(a[:], wk(kt), Act.Abs)
            ak.append(a)

        # ---- 2:4 threshold, range 0 (k-tiles 0-3): max-side on DVE,
        # min-side on GPSIMD, combine on DVE ----
        def thr_parts(r):
            a0, a1, a2, a3 = ak[4 * r : 4 * r + 4]
            tA = tmax_p.tile([P, OUT_F], f32, tag="tA", name=f"tA_{r}")
            tB = tmax_p.tile([P, OUT_F], f32, tag="tB", name=f"tB_{r}")
            tC = tmin_p.tile([P, OUT_F], f32, tag="tC", name=f"tC_{r}")
            tD = tmin_p.tile([P, OUT_F], f32, tag="tD", name=f"tD_{r}")
            vtt(tA[:], a0[:], a1[:], Alu.max)
            vtt(tB[:], a2[:], a3[:], Alu.max)
            vtt(tC[:], a0[:], a1[:], Alu.min)
            vtt(tD[:], a2[:], a3[:], Alu.min)
            return tA, tB, tC, tD

        def thr_combine(r, tA, tB, tC, tD):
            tr = thr_p.tile([P, OUT_F], f32, tag="thr", name=f"thr_{r}")
            vtt(tr[:], tA[:], tB[:], Alu.min)  # t1 = min of pair maxes
            vtt(tB[:], tC[:], tD[:], Alu.max)  # t2 = max of pair mins
            vtt(tr[:], tr[:], tB[:], Alu.max)  # thr = max(t1, t2)
            return tr

        thr0_parts = thr_parts(0)

        # ---- global absmax -> s = absmax/qmax, inv ~= 1/s ----
        amc = sc_p.tile([P, 1], f32, tag="amc")
        nc.vector.tensor_reduce(
            out=amc[:], in_=cm[:], axis=mybir.AxisListType.X, op=Alu.max
        )
        am = sc_p.tile([P, 1], f32, tag="am")
        nc.gpsimd.partition_all_reduce(
            am[:], amc[:], channels=P, reduce_op=bass_isa.ReduceOp.max
        )
        s_t = sc_p.tile([P, 1], f32, tag="s")
        vts(s_t[:], am[:], inv_qmax, Alu.mult)
        # reciprocal + one Newton-Raphson step: rel err ~2^-22 -> a handful of
        # +/-1 flips of q across the whole weight (harmless at 2e-2 tolerance)
        r0 = sc_p.tile([P, 1], f32, tag="r0")
        nc.vector.reciprocal(r0[:], s_t[:])
        p1 = sc_p.tile([P, 1], f32, tag="p1")
        e1 = sc_p.tile([P, 1], f32, tag="e1")
        r1 = sc_p.tile([P, 1], f32, tag="r1")
        inv_t = sc_p.tile([P, 1], f32, tag="inv")
        vtt(p1[:], s_t[:], r0[:], Alu.mult)
        vts(e1[:], p1[:], 2.0, Alu.subtract)  # p1 - 2 = -(2 - p1)
        vtt(r1[:], r0[:], e1[:], Alu.mult)
        vts(inv_t[:], r1[:], -1.0, Alu.mult)  # r0 * (2 - p1)

        thr0 = thr_combine(0, *thr0_parts)

        # ---- quantize per k-tile: range 0 first, then range 1.
        # q0 (magic-rne of w*inv) on ACT for k-tiles 0-3, GPSIMD for 4-7;
        # unmagic+f16 on ACT; mask and mask-multiply on DVE.
        wqt = [None] * K_TILES
        thr_by_r = {0: thr0}

        def quant(kt):
            r = kt // 4
            tr = thr_by_r[r]
            m16 = m_p.tile([P, OUT_F], f16, tag="mask")
            vtt(m16[:], ak[kt][:], tr[:], Alu.is_ge)  # 0.0/1.0, exact in fp16
            q0 = q0_p.tile([P, OUT_F], f32, tag="q0")
            nc.scalar.activation(
                q0[:], wk(kt), Act.Identity, bias=magic_t[:], scale=inv_t[:]
            )
            q16u = q0_p.tile([P, OUT_F], f16, tag="q16u")
            nc.scalar.activation(
                q16u[:], q0[:], Act.Identity, bias=nmagic_t[:], scale=one_t[:]
            )
            q16 = q16_p.tile([P, OUT_F], f16, tag="q16", name=f"q16_{kt}")
            vtt(q16[:], q16u[:], m16[:], Alu.mult)  # apply 2:4 mask
            wqt[kt] = q16

        for kt in (0, 1, 2, 3):
            quant(kt)
        thr1_parts = thr_parts(1)
        thr_by_r[1] = thr_combine(1, *thr1_parts)
        for kt in (4, 5, 6, 7):
            quant(kt)

        # ---- x loads: per (k-tile, token-half), 0.5MB each, both queues ----
        xh = [[None] * K_TILES, [None] * K_TILES]
        for h in range(2):
            for kt in range(K_TILES):
                t = x_p.tile([P, TOK_H], f16, tag="x", name=f"x{h}_{kt}")
                eng = nc.sync if kt < 4 else nc.scalar
                eng.dma_start(t[:], xt16[:, kt, h * TOK_H : (h + 1) * TOK_H])
                xh[h][kt] = t

        # ---- phase A: tokens 0:512, k-outer over all 8 PSUM banks ----
        psA = [
            psum_mm.tile([P, MM_N], f32, tag="ps", name=f"psA_{mi}")
            for mi in range(M_TILES)
        ]
        for kt in range(K_TILES):
            if kt == 4:
                # range-1 q16 tiles trail the DVE threshold chain; bridge the
                # PE hole with zero-weight matmuls (add 0 to live banks) so
                # HAM stays warm instead of re-throttling across the wait
                for zi in range(16):
                    nc.tensor.matmul(
                        psA[zi % M_TILES][:],
                        z16[:],
                        xh[0][0][:, 0:TOK_A],
                        start=False,
                        stop=False,
                    )
            for mi in range(M_TILES):
                nc.tensor.matmul(
                    psA[mi][:],
                    wqt[kt][:, mi * P : (mi + 1) * P],
                    xh[0][kt][:, 0:TOK_A],
                    start=(kt == 0),
                    stop=(kt == K_TILES - 1),
                )
        for mi in range(M_TILES):
            ya = ya_p.tile([P, TOK_A], f16, tag="ya", name=f"yA_{mi}")
            nc.scalar.activation(
                ya[:], psA[mi][:], Act.Identity, bias=bias_t[mi][:], scale=s_t[:]
            )
            eng = nc.sync if mi % 2 == 0 else nc.scalar
            eng.dma_start(yt[mi * P : (mi + 1) * P, 0:TOK_A], ya[:])

        # ---- phases P1/P2: m-outer, stationary weight reused over banks ----
        # P1 = tokens 512:2048 (3 banks / m-tile), P2 = 2048:4096 (4 banks)
        for phase, (h, x0, ncols) in enumerate(
            ((0, TOK_A, TOK_H - TOK_A), (1, 0, TOK_H))
        ):
            ntj = ncols // MM_N
            col0 = h * TOK_H + x0
            for mi in range(M_TILES):
                ps = [
                    psum_mm.tile([P, MM_N], f32, tag="ps", name=f"psB{phase}_{mi}_{tj}")
                    for tj in range(ntj)
                ]
                for kt in range(K_TILES):
                    lhsT = wqt[kt][:, mi * P : (mi + 1) * P]
                    for tj in range(ntj):
                        nc.tensor.matmul(
                            ps[tj][:],
                            lhsT,
                            xh[h][kt][:, x0 + tj * MM_N : x0 + (tj + 1) * MM_N],
                            start=(kt == 0),
                            stop=(kt == K_TILES - 1),
                        )
                yb = yb_p.tile([P, ncols], f16, tag="yb", name=f"yB{phase}_{mi}")
                for tj in range(ntj):
                    nc.scalar.activation(
                        yb[:, tj * MM_N : (tj + 1) * MM_N],
                        ps[tj][:],
                        Act.Identity,
                        bias=bias_t[mi][:],
                        scale=s_t[:],
                    )
                eng = nc.sync if mi % 2 == 0 else nc.scalar
                eng.dma_start(
                    yt[mi * P : (mi + 1) * P, col0 : col0 + ncols], yb[:]
                )

    nc.compile()
    return nc


def _get(qmax: float):
    key = qmax
    if key not in _CACHE:
        _CACHE[key] = _build(qmax)
    return _CACHE[key]


def host_prep(x, weight):
    """Host-side input re-encoding: transpose, phase-major permute the in_f
    axis, partition-major re-layout, fp16 cast of x. No module math."""
    xt = np.ascontiguousarray(x.T)[_PERM].astype(np.float16)  # [IN_F, TOKENS]
    xm = np.ascontiguousarray(
        xt.reshape(K_TILES, P, TOKENS).transpose(1, 0, 2)
    )  # [P, K_TILES, TOKENS]
    wt = np.ascontiguousarray(weight.T)[_PERM]  # [IN_F, OUT_F]
    wm = np.ascontiguousarray(
        wt.reshape(K_TILES, P, OUT_F).transpose(1, 0, 2).reshape(P, K_TILES * OUT_F)
    )
    return xm, wm


LAST_EXEC_NS = None


def kernel(x, weight, bias, precision, _trace_dir=None):
    global LAST_EXEC_NS
    from concourse.bass_utils import run_bass_kernel_spmd

    x = np.asarray(x, dtype=np.float32)
    weight = np.asarray(weight, dtype=np.float32)
    bias = np.asarray(bias, dtype=np.float32)
    prec = int(np.asarray(precision))
    qmax = float(2 ** (prec - 1) - 1)

    nc = _get(qmax)

    xm, wm = host_prep(x, weight)
    in_maps = [
        {
            "xt16": np.ascontiguousarray(
                xm[:, :, c * TOK_PER_CORE : (c + 1) * TOK_PER_CORE]
            ),
            "wp": wm,
            "bias": bias,
        }
        for c in range(N_CORES)
    ]
    kw = {}
    if _trace_dir is not None:
        kw = {"trace": True, "tmpdir": _trace_dir}
    res = run_bass_kernel_spmd(nc, in_maps, list(range(N_CORES)), **kw)
    LAST_EXEC_NS = res.exec_time_ns
    yt = np.concatenate([res.results[c]["yt"] for c in range(N_CORES)], axis=1)
    return np.ascontiguousarray(yt.T).astype(np.float32)
